# revision 11
# baseline (speedup 1.0000x reference)
# Trainium2 Bass kernel for EndPointRepr (span endpoint representations).
#
# reference:
#   h = encoded_input @ W + b                    # [B, S, P]
#   res_k[q] = concat(h[qb[q], s_k[q]], h[qb[q], e_k[q]]) * (e_k[q] >= s_k[q])
#
# Sharding: data-parallel over batch. Core c owns batch c; the host routes
# each valid (e >= s) query to its batch's core. Invalid queries are never
# routed; the host-side result buffers start zeroed.
#
# Device pipeline (bf16 data path, fp32 PSUM accumulation):
#   The host compacts the batch to only the h rows some query references
#   (~1350 of 2048, capacity HROWS), remaps the indices, and sorts each
#   stream's queries by max referenced row. All inputs are host-packed for
#   contiguous large-descriptor DMAs.
#   phase 1: X chunks stream in split across the two HWDGE queues (sync +
#            scalar, loads issued up-front so nothing blocks a queue head);
#            per 128-row block, 8 k-block matmuls accumulate h in PSUM; DVE
#            folds the bias while down-casting to bf16; the Pool queue
#            spills each block to a DRAM h scratch. Identity warmup matmuls
#            ramp the PE p-state during the NEFF preamble.
#   phase 2: per 128-query tile, a software-DGE indirect DMA (one dynamic
#            row offset per partition) gathers endpoint rows DRAM->SBUF;
#            emission is interleaved with the spills in bound order so the
#            serial SWDGE chain starts while the matmul still runs. A plain
#            DMA per tile writes the natural [QCAP, 2P] result slices.
# bf16 keeps the PE at 1 cycle/row (fp32 is 4) and halves all DMA traffic;
# rel err ~3e-3 against the fp32 reference, well inside the 2e-2 gate.
import numpy as np

B, S, D, P = 8, 2048, 1024, 256
NQ = 8192
NCORES = 8
KB = D // 128          # contraction k-blocks
HROWS = 1536           # compacted h row capacity (multiple of SCHW)
HB = HROWS // 128      # h row blocks
QCAP = 640             # per-endpoint query capacity (multiple of 128)
QT = QCAP // 128       # query tiles per stream-endpoint
SCH = 4                # x chunks for DMA/matmul overlap
SCHW = HROWS // SCH
NWARM = 32             # PE warmup matmuls (p-state ramp during preamble)
PROBE = True           # emit a [128, 2]-offset indirect-DMA probe

_cache = {}


def _build_nc():
    import concourse.bacc as bacc
    import concourse.mybir as mybir
    import concourse.tile as tile
    from concourse.masks import make_identity
    from concourse.tile import add_dep_helper
    from concourse.bass import IndirectOffsetOnAxis

    f32 = mybir.dt.float32
    bf16 = mybir.dt.bfloat16
    nc = bacc.Bacc("TRN2", target_bir_lowering=False, debug=False,
                   num_devices=NCORES)

    xh = nc.dram_tensor("xh", [128, SCH * KB * SCHW], bf16,
                        kind="ExternalInput").ap()
    wh = nc.dram_tensor("wh", [128, KB * P], bf16, kind="ExternalInput").ap()
    bias = nc.dram_tensor("bias", [128, P], f32, kind="ExternalInput").ap()
    off = nc.dram_tensor("off", [128, 4 * QT], mybir.dt.int32,
                         kind="ExternalInput").ap()
    r1 = nc.dram_tensor("r1", [QCAP, 2 * P], bf16, kind="ExternalOutput").ap()
    r2 = nc.dram_tensor("r2", [QCAP, 2 * P], bf16, kind="ExternalOutput").ap()
    probe = nc.dram_tensor("probe", [128, 2 * P], bf16,
                           kind="ExternalOutput").ap() if PROBE else None
    h_dram = nc.dram_tensor("h_scratch", [HROWS, P], bf16).ap()

    # host-computed spill-block prefix needed per (stream-endpoint, tile)
    mb_need = _cache["mb_need"]  # [4 * QT]

    with tile.TileContext(nc) as tc:
        with (
            tc.tile_pool(name="consts", bufs=1) as consts,
            tc.tile_pool(name="xin", bufs=SCH) as xt_pool,
            tc.tile_pool(name="hsb", bufs=3) as h_pool,
            tc.tile_pool(name="gath", bufs=8) as g_pool,
            tc.tile_pool(name="ps", bufs=4, space="PSUM") as ps_pool,
            tc.tile_pool(name="warm", bufs=2, space="PSUM") as warm_pool,
        ):
            identity = consts.tile([128, 128], bf16)
            make_identity(nc, identity)
            warm_tiles = [warm_pool.tile([128, 128], f32, name=f"w{i}")
                          for i in range(2)]
            for i in range(NWARM):
                nc.tensor.matmul(warm_tiles[i % 2], identity, identity,
                                 start=True, stop=True)

            w_sb = consts.tile([128, KB, P], bf16)
            nc.scalar.dma_start(w_sb,
                                wh.rearrange("p (kb j) -> p kb j", kb=KB))
            bias_sb = consts.tile([128, P], f32)
            nc.scalar.dma_start(bias_sb, bias)
            off_sb = consts.tile([128, 4 * QT], mybir.dt.int32)
            nc.scalar.dma_start(off_sb, off)

            xh_view = xh.rearrange("p (c kb s) -> p c kb s", c=SCH, kb=KB)
            HW = SCHW // 2
            xt_tiles = [xt_pool.tile([128, KB, SCHW], bf16, name=f"xt{c}")
                        for c in range(SCH)]
            for c in range(SCH):
                nc.sync.dma_start(xt_tiles[c][:, :, 0:HW],
                                  xh_view[:, c, :, 0:HW])
            for c in range(SCH):
                nc.scalar.dma_start(xt_tiles[c][:, :, HW:SCHW],
                                    xh_view[:, c, :, HW:SCHW])

            # gather tiles in ascending bound order, interleaved with spills
            order = sorted(range(4 * QT), key=lambda j: mb_need[j])
            h_writes = []
            gq = 0  # next gather (in `order`) to emit

            def emit_gathers(avail):
                nonlocal gq
                while gq < len(order) and mb_need[order[gq]] <= avail:
                    j = order[gq]
                    st, t = j // (2 * QT), j % (2 * QT)
                    r = r1 if st == 0 else r2
                    endp, tl = t // QT, t % QT  # endpoint, local tile
                    g_sb = g_pool.tile([128, 1, P], bf16, tag="g")
                    gi = nc.gpsimd.indirect_dma_start(
                        out=g_sb[:, 0, :],
                        out_offset=None,
                        in_=h_dram[:, :],
                        in_offset=IndirectOffsetOnAxis(
                            ap=off_sb[:, j:j + 1], axis=0),
                    )
                    for m in range(mb_need[j]):
                        add_dep_helper(gi.ins, h_writes[m].ins,
                                       reason=f"gather {j} reads h")
                    out_view = r.rearrange("(t p) c -> p t c", p=128)
                    nc.sync.dma_start(
                        out_view[:, tl:tl + 1, endp * P:(endp + 1) * P],
                        g_sb)
                    gq += 1

            for c in range(SCH):
                for ml in range(SCHW // 128):
                    m = c * (SCHW // 128) + ml
                    h_ps = ps_pool.tile([128, P], f32, tag="hps")
                    for kb in range(KB):
                        nc.tensor.matmul(
                            h_ps,
                            xt_tiles[c][:, kb, ml * 128:(ml + 1) * 128],
                            w_sb[:, kb, :],
                            start=(kb == 0), stop=(kb == KB - 1))
                    h_sb = h_pool.tile([128, P], bf16, tag="h")
                    nc.vector.tensor_add(h_sb, h_ps, bias_sb)
                    h_writes.append(
                        nc.gpsimd.dma_start(
                            h_dram[m * 128:(m + 1) * 128, :], h_sb))
                    emit_gathers(m + 1)
            emit_gathers(HB)
            assert gq == len(order)

            if PROBE:
                # multi-offset probe: 2 offsets per partition from off cols
                # [0, 1] (stream1-s tiles 0 and 1); result layout tells us
                # the hardware's offset walk order (or that it's garbage).
                gp = g_pool.tile([128, 2, P], bf16, name="gprobe")
                gi = nc.gpsimd.indirect_dma_start(
                    out=gp[:, :, :],
                    out_offset=None,
                    in_=h_dram[:, :],
                    in_offset=IndirectOffsetOnAxis(
                        ap=off_sb[:, 0:2], axis=0),
                )
                for m in range(HB):
                    add_dep_helper(gi.ins, h_writes[m].ins,
                                   reason="probe reads h")
                nc.sync.dma_start(
                    probe.rearrange("p (u c) -> p u c", u=2), gp)

    nc.compile()
    return nc


def _get_nc(mb_need):
    key = ("nc", tuple(mb_need))
    if key not in _cache:
        _cache["mb_need"] = list(mb_need)
        _cache[key] = _build_nc()
    return _cache[key]


def _numpy_ref(flag, encoded_input, start_ids_1, end_ids_1, query_batch_idx,
               start_ids_2, end_ids_2, W, b):
    h = encoded_input.astype(np.float32) @ W.astype(np.float32) + \
        b.astype(np.float32)
    qb = np.asarray(query_batch_idx).astype(np.int64)

    def span(s, e):
        s = np.asarray(s).astype(np.int64)
        e = np.asarray(e).astype(np.int64)
        rep = np.concatenate([h[qb, s], h[qb, e]], axis=-1)
        return rep * (e >= s)[:, None].astype(rep.dtype)

    return span(start_ids_1, end_ids_1), span(start_ids_2, end_ids_2)


def kernel(flag, encoded_input, start_ids_1, end_ids_1, query_batch_idx,
           start_ids_2, end_ids_2, W, b):
    import ml_dtypes
    from concourse.bass_utils import run_bass_kernel_spmd

    bf16 = ml_dtypes.bfloat16
    x_full = np.asarray(encoded_input, dtype=np.float32)
    w_np = np.asarray(W, dtype=np.float32)
    b_np = np.asarray(b).astype(np.float32)
    qb = np.asarray(query_batch_idx).astype(np.int64)
    s1 = np.asarray(start_ids_1).astype(np.int64)
    e1 = np.asarray(end_ids_1).astype(np.int64)
    s2 = np.asarray(start_ids_2).astype(np.int64)
    e2 = np.asarray(end_ids_2).astype(np.int64)

    in_range = (qb.min() >= 0 and qb.max() < B and
                all(a.min() >= 0 and a.max() < S for a in (s1, e1, s2, e2)))

    in_maps, ids_all = [], []
    mb_need = np.ones(4 * QT, np.int64)
    try:
        if not in_range or x_full.shape != (B, S, D):
            raise ValueError("shape/range")
        wh = np.ascontiguousarray(
            w_np.reshape(KB, 128, P).transpose(1, 0, 2).reshape(128, KB * P)
        ).astype(bf16)
        bias_rep = np.ascontiguousarray(
            np.broadcast_to(b_np[None, :], (128, P)), dtype=np.float32)
        cdata = []
        for bb in range(B):
            sel = qb == bb
            ids1 = np.nonzero(sel & (e1 >= s1))[0]
            ids2 = np.nonzero(sel & (e2 >= s2))[0]
            if len(ids1) > QCAP or len(ids2) > QCAP:
                raise ValueError("capacity overflow")
            rows = np.unique(np.concatenate(
                [s1[ids1], e1[ids1], s2[ids2], e2[ids2],
                 np.zeros(1, np.int64)]))
            if len(rows) > HROWS:
                raise ValueError("row overflow")
            cs1 = np.searchsorted(rows, s1[ids1]).astype(np.int32)
            ce1 = np.searchsorted(rows, e1[ids1]).astype(np.int32)
            cs2 = np.searchsorted(rows, s2[ids2]).astype(np.int32)
            ce2 = np.searchsorted(rows, e2[ids2]).astype(np.int32)
            o1 = np.argsort(np.maximum(cs1, ce1), kind="stable")
            o2 = np.argsort(np.maximum(cs2, ce2), kind="stable")
            ids1, cs1, ce1 = ids1[o1], cs1[o1], ce1[o1]
            ids2, cs2, ce2 = ids2[o2], cs2[o2], ce2[o2]
            ids_all.append((ids1, ids2))
            off_np = np.zeros((4, QCAP), np.int32)
            for j, carr in enumerate([cs1, ce1, cs2, ce2]):
                off_np[j, :len(carr)] = carr
            for st_ep in range(4):
                for t in range(QT):
                    seg = off_np[st_ep, t * 128:(t + 1) * 128]
                    mb = int(seg.max()) // 128 + 1
                    # global j index: stream st = st_ep // 2,
                    # endpoint = st_ep % 2 -> j = st*(2QT) + endp*QT + t
                    st, endp = st_ep // 2, st_ep % 2
                    j = st * 2 * QT + endp * QT + t
                    mb_need[j] = max(mb_need[j], mb)
            cdata.append((rows, off_np))
        for bb in range(B):
            rows, off_np = cdata[bb]
            # off[p, j]: j = st*2*QT + endp*QT + t, slot q = t*128 + p
            off_w = np.zeros((128, 4 * QT), np.int32)
            for st_ep in range(4):
                st, endp = st_ep // 2, st_ep % 2
                for t in range(QT):
                    j = st * 2 * QT + endp * QT + t
                    off_w[:, j] = off_np[st_ep, t * 128:(t + 1) * 128]
            xc = np.zeros((HROWS, D), np.float32)
            xc[:len(rows)] = x_full[bb][rows]
            xr = xc.reshape(SCH, SCHW, KB, 128).transpose(3, 0, 2, 1) \
                .reshape(128, SCH * KB * SCHW)
            in_maps.append({
                "xh": np.ascontiguousarray(xr).astype(bf16),
                "wh": wh,
                "bias": bias_rep,
                "off": np.ascontiguousarray(off_w),
            })
    except ValueError:
        res1, res2 = _numpy_ref(flag, x_full, s1, e1, qb, s2, e2, w_np, b_np)
        return np.asarray(res1, np.float32), np.asarray(res2, np.float32)

    nc = _get_nc(tuple(int(v) for v in mb_need))
    out = run_bass_kernel_spmd(nc, in_maps, core_ids=list(range(NCORES)))
    _cache["last_run"] = out

    res1 = np.zeros((NQ, 2 * P), np.float32)
    res2 = np.zeros((NQ, 2 * P), np.float32)
    for bb in range(B):
        for ids, res, key in [(ids_all[bb][0], res1, "r1"),
                              (ids_all[bb][1], res2, "r2")]:
            n = len(ids)
            if n:
                r = np.asarray(out.results[bb][key]).astype(np.float32)
                res[ids] = r[:n]
    return res1, res2


# revision 12
# speedup vs baseline: 1.2839x; 1.2839x over previous
# Trainium2 Bass kernel for EndPointRepr (span endpoint representations).
#
# reference:
#   h = encoded_input @ W + b                    # [B, S, P]
#   res_k[q] = concat(h[qb[q], s_k[q]], h[qb[q], e_k[q]]) * (e_k[q] >= s_k[q])
#
# Sharding: data-parallel over batch. Core c owns batch c; the host routes
# each valid (e >= s) query to its batch's core. Invalid queries are never
# routed; the host-side result buffers start zeroed.
#
# Device pipeline (bf16 data path, fp32 PSUM accumulation):
#   The host compacts the batch to only the h rows some query references
#   (~1350 of 2048, capacity HROWS). Queries are sorted by max referenced
#   row and compact row ids are assigned in FIRST-USE order of that global
#   sequence, so the rows a query tile needs grow linearly with tile index
#   and gathers can chase the matmul block by block.
#   phase 1: X chunks stream in, k-halves split across the two HWDGE queues
#            (sync + scalar, issued up-front); per 128-row block, 8 k-block
#            matmuls accumulate h in PSUM; DVE folds the bias while down-
#            casting to bf16; sync spills each block to a DRAM h scratch.
#            Identity warmup matmuls ramp the PE p-state in the preamble.
#   phase 2: per 128-query tile, a software-DGE indirect DMA (one dynamic
#            row offset per partition, the only HW-supported form) gathers
#            endpoint rows DRAM->SBUF on the otherwise-empty Pool ring,
#            emitted in bound order interleaved with phase 1 so the serial
#            SWDGE chain starts as soon as its h prefix has landed; scalar
#            writes each tile into the natural [QCAP, 2P] result.
# bf16 keeps the PE at 1 cycle/row (fp32 is 4) and halves all DMA traffic;
# rel err ~3e-3 against the fp32 reference, well inside the 2e-2 gate.
import numpy as np

B, S, D, P = 8, 2048, 1024, 256
NQ = 8192
NCORES = 8
KB = D // 128          # contraction k-blocks
HROWS = 1536           # compacted h row capacity (multiple of SCHW)
HB = HROWS // 128      # h row blocks
QCAP = 640             # per-endpoint query capacity (multiple of 128)
QT = QCAP // 128       # query tiles per stream-endpoint
SCH = 4                # x chunks for DMA/matmul overlap
SCHW = HROWS // SCH
NWARM = 32             # PE warmup matmuls (p-state ramp during preamble)

_cache = {}


def _build_nc():
    import concourse.bacc as bacc
    import concourse.mybir as mybir
    import concourse.tile as tile
    from concourse.masks import make_identity
    from concourse.tile import add_dep_helper
    from concourse.bass import IndirectOffsetOnAxis

    f32 = mybir.dt.float32
    bf16 = mybir.dt.bfloat16
    nc = bacc.Bacc("TRN2", target_bir_lowering=False, debug=False,
                   num_devices=NCORES)

    xh = nc.dram_tensor("xh", [128, SCH * KB * SCHW], bf16,
                        kind="ExternalInput").ap()
    wh = nc.dram_tensor("wh", [128, KB * P], bf16, kind="ExternalInput").ap()
    bias = nc.dram_tensor("bias", [128, P], f32, kind="ExternalInput").ap()
    off = nc.dram_tensor("off", [128, 4 * QT], mybir.dt.int32,
                         kind="ExternalInput").ap()
    r1 = nc.dram_tensor("r1", [QCAP, 2 * P], bf16, kind="ExternalOutput").ap()
    r2 = nc.dram_tensor("r2", [QCAP, 2 * P], bf16, kind="ExternalOutput").ap()
    h_dram = nc.dram_tensor("h_scratch", [HROWS, P], bf16).ap()

    mb_need = _cache["mb_need"]  # [4 * QT] spill-block prefix per tile

    with tile.TileContext(nc) as tc:
        with (
            tc.tile_pool(name="consts", bufs=1) as consts,
            tc.tile_pool(name="xin", bufs=SCH) as xt_pool,
            tc.tile_pool(name="hsb", bufs=1) as h_pool,
            tc.tile_pool(name="gath", bufs=8) as g_pool,
            tc.tile_pool(name="ps", bufs=4, space="PSUM") as ps_pool,
            tc.tile_pool(name="warm", bufs=2, space="PSUM") as warm_pool,
        ):
            identity = consts.tile([128, 128], bf16)
            make_identity(nc, identity)
            warm_tiles = [warm_pool.tile([128, 128], f32, name=f"w{i}")
                          for i in range(2)]
            for i in range(NWARM):
                nc.tensor.matmul(warm_tiles[i % 2], identity, identity,
                                 start=True, stop=True)

            w_sb = consts.tile([128, KB, P], bf16)
            nc.scalar.dma_start(w_sb,
                                wh.rearrange("p (kb j) -> p kb j", kb=KB))
            bias_sb = consts.tile([128, P], f32)
            nc.scalar.dma_start(bias_sb, bias)
            off_sb = consts.tile([128, 4 * QT], mybir.dt.int32)
            nc.scalar.dma_start(off_sb, off)

            xh_view = xh.rearrange("p (c kb s) -> p c kb s", c=SCH, kb=KB)
            KH = KB // 2
            xt_tiles = [xt_pool.tile([128, KB, SCHW], bf16, name=f"xt{c}")
                        for c in range(SCH)]
            for c in range(SCH):
                nc.sync.dma_start(xt_tiles[c][:, 0:KH, :],
                                  xh_view[:, c, 0:KH, :])
            for c in range(SCH):
                nc.scalar.dma_start(xt_tiles[c][:, KH:KB, :],
                                    xh_view[:, c, KH:KB, :])

            order = sorted(range(4 * QT), key=lambda j: mb_need[j])
            h_writes = []
            gq = 0

            def emit_gathers(avail):
                nonlocal gq
                while gq < len(order) and mb_need[order[gq]] <= avail:
                    j = order[gq]
                    st, t = j // (2 * QT), j % (2 * QT)
                    r = r1 if st == 0 else r2
                    endp, tl = t // QT, t % QT
                    g_sb = g_pool.tile([128, 1, P], bf16, tag="g")
                    gi = nc.gpsimd.indirect_dma_start(
                        out=g_sb[:, 0, :],
                        out_offset=None,
                        in_=h_dram[:, :],
                        in_offset=IndirectOffsetOnAxis(
                            ap=off_sb[:, j:j + 1], axis=0),
                    )
                    for m in range(mb_need[j]):
                        add_dep_helper(gi.ins, h_writes[m].ins,
                                       reason=f"gather {j} reads h")
                    out_view = r.rearrange("(t p) c -> p t c", p=128)
                    nc.scalar.dma_start(
                        out_view[:, tl:tl + 1, endp * P:(endp + 1) * P],
                        g_sb)
                    gq += 1

            for c in range(SCH):
                for ml in range(SCHW // 128):
                    m = c * (SCHW // 128) + ml
                    h_ps = ps_pool.tile([128, P], f32, tag="hps")
                    for kb in range(KB):
                        nc.tensor.matmul(
                            h_ps,
                            xt_tiles[c][:, kb, ml * 128:(ml + 1) * 128],
                            w_sb[:, kb, :],
                            start=(kb == 0), stop=(kb == KB - 1))
                    h_sb = h_pool.tile([128, P], bf16, name=f"h{m}")
                    nc.vector.tensor_add(h_sb, h_ps, bias_sb)
                    h_writes.append(
                        nc.sync.dma_start(
                            h_dram[m * 128:(m + 1) * 128, :], h_sb))
                    emit_gathers(m + 1)
            emit_gathers(HB)
            assert gq == len(order)

    nc.compile()
    return nc


def _get_nc(mb_need):
    key = ("nc", tuple(mb_need))
    if key not in _cache:
        _cache["mb_need"] = list(mb_need)
        _cache[key] = _build_nc()
    return _cache[key]


def _numpy_ref(flag, encoded_input, start_ids_1, end_ids_1, query_batch_idx,
               start_ids_2, end_ids_2, W, b):
    h = encoded_input.astype(np.float32) @ W.astype(np.float32) + \
        b.astype(np.float32)
    qb = np.asarray(query_batch_idx).astype(np.int64)

    def span(s, e):
        s = np.asarray(s).astype(np.int64)
        e = np.asarray(e).astype(np.int64)
        rep = np.concatenate([h[qb, s], h[qb, e]], axis=-1)
        return rep * (e >= s)[:, None].astype(rep.dtype)

    return span(start_ids_1, end_ids_1), span(start_ids_2, end_ids_2)


def kernel(flag, encoded_input, start_ids_1, end_ids_1, query_batch_idx,
           start_ids_2, end_ids_2, W, b):
    import ml_dtypes
    from concourse.bass_utils import run_bass_kernel_spmd

    bf16 = ml_dtypes.bfloat16
    x_full = np.asarray(encoded_input, dtype=np.float32)
    w_np = np.asarray(W, dtype=np.float32)
    b_np = np.asarray(b).astype(np.float32)
    qb = np.asarray(query_batch_idx).astype(np.int64)
    s1 = np.asarray(start_ids_1).astype(np.int64)
    e1 = np.asarray(end_ids_1).astype(np.int64)
    s2 = np.asarray(start_ids_2).astype(np.int64)
    e2 = np.asarray(end_ids_2).astype(np.int64)

    in_range = (qb.min() >= 0 and qb.max() < B and
                all(a.min() >= 0 and a.max() < S for a in (s1, e1, s2, e2)))

    in_maps, ids_all = [], []
    mb_need = np.ones(4 * QT, np.int64)
    try:
        if not in_range or x_full.shape != (B, S, D):
            raise ValueError("shape/range")
        wh = np.ascontiguousarray(
            w_np.reshape(KB, 128, P).transpose(1, 0, 2).reshape(128, KB * P)
        ).astype(bf16)
        bias_rep = np.ascontiguousarray(
            np.broadcast_to(b_np[None, :], (128, P)), dtype=np.float32)
        cdata = []
        for bb in range(B):
            sel = qb == bb
            ids1 = np.nonzero(sel & (e1 >= s1))[0]
            ids2 = np.nonzero(sel & (e2 >= s2))[0]
            if len(ids1) > QCAP or len(ids2) > QCAP:
                raise ValueError("capacity overflow")
            # global pair sequence sorted by max(s, e); first-use row ids
            n1, n2 = len(ids1), len(ids2)
            allmax = np.concatenate([np.maximum(s1[ids1], e1[ids1]),
                                     np.maximum(s2[ids2], e2[ids2])])
            gord = np.argsort(allmax, kind="stable")
            # row sequence in that order: (s, e) per pair
            alls = np.concatenate([s1[ids1], s2[ids2]])[gord]
            alle = np.concatenate([e1[ids1], e2[ids2]])[gord]
            seq = np.empty(2 * len(gord), np.int64)
            seq[0::2] = alls
            seq[1::2] = alle
            seq = np.concatenate([np.zeros(1, np.int64), seq])
            uniq, first = np.unique(seq, return_index=True)
            if len(uniq) > HROWS:
                raise ValueError("row overflow")
            # compact id = rank of first occurrence
            fo = np.argsort(first, kind="stable")
            rows = uniq[fo]                       # original row per compact id
            rank = np.empty(len(uniq), np.int64)
            rank[fo] = np.arange(len(uniq))
            # compact ids for each stream (uniq is sorted by value)
            def cmp(a):
                return rank[np.searchsorted(uniq, a)].astype(np.int32)
            cs1, ce1 = cmp(s1[ids1]), cmp(e1[ids1])
            cs2, ce2 = cmp(s2[ids2]), cmp(e2[ids2])
            # sort each stream's queries by max compact row
            o1 = np.argsort(np.maximum(cs1, ce1), kind="stable")
            o2 = np.argsort(np.maximum(cs2, ce2), kind="stable")
            ids1, cs1, ce1 = ids1[o1], cs1[o1], ce1[o1]
            ids2, cs2, ce2 = ids2[o2], cs2[o2], ce2[o2]
            ids_all.append((ids1, ids2))
            off_np = np.zeros((4, QCAP), np.int32)
            for j, carr in enumerate([cs1, ce1, cs2, ce2]):
                off_np[j, :len(carr)] = carr
            for st_ep in range(4):
                st, endp = st_ep // 2, st_ep % 2
                for t in range(QT):
                    seg = off_np[st_ep, t * 128:(t + 1) * 128]
                    j = st * 2 * QT + endp * QT + t
                    mb_need[j] = max(mb_need[j], int(seg.max()) // 128 + 1)
            cdata.append((rows, off_np))
        for bb in range(B):
            rows, off_np = cdata[bb]
            off_w = np.zeros((128, 4 * QT), np.int32)
            for st_ep in range(4):
                st, endp = st_ep // 2, st_ep % 2
                for t in range(QT):
                    j = st * 2 * QT + endp * QT + t
                    off_w[:, j] = off_np[st_ep, t * 128:(t + 1) * 128]
            xc = np.zeros((HROWS, D), np.float32)
            xc[:len(rows)] = x_full[bb][rows]
            xr = xc.reshape(SCH, SCHW, KB, 128).transpose(3, 0, 2, 1) \
                .reshape(128, SCH * KB * SCHW)
            in_maps.append({
                "xh": np.ascontiguousarray(xr).astype(bf16),
                "wh": wh,
                "bias": bias_rep,
                "off": np.ascontiguousarray(off_w),
            })
    except ValueError:
        res1, res2 = _numpy_ref(flag, x_full, s1, e1, qb, s2, e2, w_np, b_np)
        return np.asarray(res1, np.float32), np.asarray(res2, np.float32)

    nc = _get_nc(tuple(int(v) for v in mb_need))
    out = run_bass_kernel_spmd(nc, in_maps, core_ids=list(range(NCORES)))
    _cache["last_run"] = out

    res1 = np.zeros((NQ, 2 * P), np.float32)
    res2 = np.zeros((NQ, 2 * P), np.float32)
    for bb in range(B):
        for ids, res, key in [(ids_all[bb][0], res1, "r1"),
                              (ids_all[bb][1], res2, "r2")]:
            n = len(ids)
            if n:
                r = np.asarray(out.results[bb][key]).astype(np.float32)
                res[ids] = r[:n]
    return res1, res2


# revision 14
# speedup vs baseline: 2.3666x; 1.8433x over previous
# Trainium2 Bass kernel for EndPointRepr (span endpoint representations).
#
# reference:
#   h = encoded_input @ W + b                    # [B, S, P]
#   res_k[q] = concat(h[qb[q], s_k[q]], h[qb[q], e_k[q]]) * (e_k[q] >= s_k[q])
#
# Sharding: data-parallel over batch. Core c owns batch c; the host routes
# each valid (e >= s) query to its batch's core. Invalid queries are never
# routed; the host-side result buffers start zeroed.
#
# Device pipeline (bf16 data path, fp32 PSUM accumulation):
#   The host compacts the batch to the h rows actually referenced (~1350 of
#   2048, capacity HROWS, ascending order) and remaps indices. Each of the
#   four endpoint streams (s1, e1, s2, e2) gets its OWN slot order sorted by
#   referenced row, so a 128-slot tile only touches a ~3-block band of h.
#   The host reassembles res from the four streams independently.
#   phase 1: X chunks stream in, k-halves split across the two HWDGE queues;
#            per 128-row block, 8 k-block matmuls accumulate h in PSUM; DVE
#            folds the bias while down-casting to bf16 SBUF tiles that stay
#            resident. Identity warmup matmuls ramp the PE p-state.
#   phase 2: gather AS MATMUL: per (stream, tile), host-built one-hot
#            selection matrices (exact 1.0 entries) multiply the resident h
#            blocks: res_tile[q, :] = sum_kb onehot[kb][:, q].T @ h[kb].
#            ACT/DVE copy PSUM to bf16 and plain DMAs write the result.
#            No GpSimd ucode, no DRAM h scratch, no indirect DMA; row
#            selection by 1.0-matmul is numerically exact.
# bf16 keeps the PE at 1 cycle/row (fp32 is 4) and halves all DMA traffic;
# rel err ~3e-3 against the fp32 reference, well inside the 2e-2 gate.
import numpy as np

B, S, D, P = 8, 2048, 1024, 256
NQ = 8192
NCORES = 8
KB = D // 128          # contraction k-blocks
HROWS = 1536           # compacted h row capacity (multiple of SCHW)
HB = HROWS // 128      # h row blocks
QCAP = 640             # per-endpoint query capacity (multiple of 128)
QT = QCAP // 128       # query tiles per endpoint stream
NST = 4                # endpoint streams: s1, e1, s2, e2
SCH = 4                # x chunks for DMA/matmul overlap
SCHW = HROWS // SCH
NWARM = 32             # PE warmup matmuls (p-state ramp during preamble)

_cache = {}


def _build_nc():
    import concourse.bacc as bacc
    import concourse.mybir as mybir
    import concourse.tile as tile
    from concourse.masks import make_identity

    f32 = mybir.dt.float32
    bf16 = mybir.dt.bfloat16
    nc = bacc.Bacc("TRN2", target_bir_lowering=False, debug=False,
                   num_devices=NCORES)

    # per-(stream, tile) k-block window [kbase, kbase+kcnt) over h blocks
    kwin = _cache["kwin"]                  # list of NST*QT (kbase, kcnt)
    bases = np.cumsum([0] + [kc for _, kc in kwin]).tolist()
    noh = bases[-1]                        # total one-hot [128,128] tiles

    xh = nc.dram_tensor("xh", [128, SCH * KB * SCHW], bf16,
                        kind="ExternalInput").ap()
    wh = nc.dram_tensor("wh", [128, KB * P], bf16, kind="ExternalInput").ap()
    bias = nc.dram_tensor("bias", [128, P], f32, kind="ExternalInput").ap()
    oh = nc.dram_tensor("oh", [128, noh * 128], bf16,
                        kind="ExternalInput").ap()
    r1 = nc.dram_tensor("r1", [QCAP, 2 * P], bf16, kind="ExternalOutput").ap()
    r2 = nc.dram_tensor("r2", [QCAP, 2 * P], bf16, kind="ExternalOutput").ap()

    with tile.TileContext(nc) as tc:
        with (
            tc.tile_pool(name="consts", bufs=1) as consts,
            tc.tile_pool(name="xin", bufs=SCH) as xt_pool,
            tc.tile_pool(name="gout", bufs=6) as g_pool,
            tc.tile_pool(name="ps", bufs=4, space="PSUM") as ps_pool,
            tc.tile_pool(name="psg", bufs=4, space="PSUM") as psg_pool,
        ):
            identity = consts.tile([128, 128], bf16)
            make_identity(nc, identity)
            for i in range(NWARM):
                warm_ps = psg_pool.tile([128, 128], f32, tag="gps")
                nc.tensor.matmul(warm_ps, identity, identity,
                                 start=True, stop=True)

            w_sb = consts.tile([128, KB, P], bf16)
            nc.scalar.dma_start(w_sb,
                                wh.rearrange("p (kb j) -> p kb j", kb=KB))
            bias_sb = consts.tile([128, P], f32)
            nc.scalar.dma_start(bias_sb, bias)

            # x chunk loads, k-halves split across the two queues, up-front
            xh_view = xh.rearrange("p (c kb s) -> p c kb s", c=SCH, kb=KB)
            KH = KB // 2
            xt_tiles = [xt_pool.tile([128, KB, SCHW], bf16, name=f"xt{c}")
                        for c in range(SCH)]
            for c in range(SCH):
                nc.sync.dma_start(xt_tiles[c][:, 0:KH, :],
                                  xh_view[:, c, 0:KH, :])
            for c in range(SCH):
                nc.scalar.dma_start(xt_tiles[c][:, KH:KB, :],
                                    xh_view[:, c, KH:KB, :])
            # one-hot tiles, halves on each queue, behind the x loads
            oh_sb = consts.tile([128, noh, 128], bf16)
            oh_view = oh.rearrange("p (i q) -> p i q", q=128)
            nh = noh // 2
            nc.sync.dma_start(oh_sb[:, 0:nh, :], oh_view[:, 0:nh, :])
            nc.scalar.dma_start(oh_sb[:, nh:noh, :], oh_view[:, nh:noh, :])

            # phase 1: h = X @ W + b, blocks stay resident in SBUF
            h_tiles = []
            for c in range(SCH):
                for ml in range(SCHW // 128):
                    m = c * (SCHW // 128) + ml
                    h_ps = ps_pool.tile([128, P], f32, tag="hps")
                    for kb in range(KB):
                        nc.tensor.matmul(
                            h_ps,
                            xt_tiles[c][:, kb, ml * 128:(ml + 1) * 128],
                            w_sb[:, kb, :],
                            start=(kb == 0), stop=(kb == KB - 1))
                    h_sb = consts.tile([128, P], bf16, name=f"h{m}")
                    nc.vector.tensor_add(h_sb, h_ps, bias_sb)
                    h_tiles.append(h_sb)

            # phase 2: gather as one-hot matmuls
            for st in range(NST):
                r = r1 if st < 2 else r2
                endp = st % 2
                out_view = r.rearrange("(t p) c -> p t c", p=128)
                for t in range(QT):
                    j = st * QT + t
                    kbase, kcnt = kwin[j]
                    g_ps = psg_pool.tile([128, P], f32, tag="gps")
                    for l in range(kcnt):
                        nc.tensor.matmul(
                            g_ps, oh_sb[:, bases[j] + l, :],
                            h_tiles[kbase + l],
                            start=(l == 0), stop=(l == kcnt - 1))
                    g_sb = g_pool.tile([128, 1, P], bf16, tag="g")
                    if (st * QT + t) % 2 == 0:
                        nc.vector.tensor_copy(g_sb[:, 0, :], g_ps)
                    else:
                        nc.scalar.copy(g_sb[:, 0, :], g_ps)
                    eng = nc.sync if st < 2 else nc.scalar
                    eng.dma_start(
                        out_view[:, t:t + 1, endp * P:(endp + 1) * P],
                        g_sb)

    nc.compile()
    return nc


def _get_nc(kwin):
    key = ("nc", tuple(kwin))
    if key not in _cache:
        _cache["kwin"] = list(kwin)
        _cache[key] = _build_nc()
    return _cache[key]


def _numpy_ref(flag, encoded_input, start_ids_1, end_ids_1, query_batch_idx,
               start_ids_2, end_ids_2, W, b):
    h = encoded_input.astype(np.float32) @ W.astype(np.float32) + \
        b.astype(np.float32)
    qb = np.asarray(query_batch_idx).astype(np.int64)

    def span(s, e):
        s = np.asarray(s).astype(np.int64)
        e = np.asarray(e).astype(np.int64)
        rep = np.concatenate([h[qb, s], h[qb, e]], axis=-1)
        return rep * (e >= s)[:, None].astype(rep.dtype)

    return span(start_ids_1, end_ids_1), span(start_ids_2, end_ids_2)


def kernel(flag, encoded_input, start_ids_1, end_ids_1, query_batch_idx,
           start_ids_2, end_ids_2, W, b):
    import ml_dtypes
    from concourse.bass_utils import run_bass_kernel_spmd

    bf16 = ml_dtypes.bfloat16
    x_full = np.asarray(encoded_input, dtype=np.float32)
    w_np = np.asarray(W, dtype=np.float32)
    b_np = np.asarray(b).astype(np.float32)
    qb = np.asarray(query_batch_idx).astype(np.int64)
    s1 = np.asarray(start_ids_1).astype(np.int64)
    e1 = np.asarray(end_ids_1).astype(np.int64)
    s2 = np.asarray(start_ids_2).astype(np.int64)
    e2 = np.asarray(end_ids_2).astype(np.int64)

    in_range = (qb.min() >= 0 and qb.max() < B and
                all(a.min() >= 0 and a.max() < S for a in (s1, e1, s2, e2)))

    percore = []
    try:
        if not in_range or x_full.shape != (B, S, D):
            raise ValueError("shape/range")
        for bb in range(B):
            sel = qb == bb
            ids1 = np.nonzero(sel & (e1 >= s1))[0]
            ids2 = np.nonzero(sel & (e2 >= s2))[0]
            if len(ids1) > QCAP or len(ids2) > QCAP:
                raise ValueError("capacity overflow")
            rows = np.unique(np.concatenate(
                [s1[ids1], e1[ids1], s2[ids2], e2[ids2]]))
            if len(rows) > HROWS:
                raise ValueError("row overflow")
            # per endpoint-stream: slot ids sorted by referenced row
            streams = []
            for ids, a in [(ids1, s1), (ids1, e1), (ids2, s2), (ids2, e2)]:
                cr = np.searchsorted(rows, a[ids]).astype(np.int64)
                o = np.argsort(cr, kind="stable")
                streams.append((ids[o], cr[o]))
            percore.append((rows, streams))
        # merged k-windows per (stream, tile) across cores
        kwin = []
        for st in range(NST):
            for t in range(QT):
                lo, hi = HB - 1, 0
                for bb in range(B):
                    cr = percore[bb][1][st][1]
                    seg = cr[t * 128:(t + 1) * 128]
                    if len(seg):
                        lo = min(lo, int(seg[0]) // 128)
                        hi = max(hi, int(seg[-1]) // 128)
                if hi < lo:
                    lo, hi = 0, 0
                kwin.append((lo, hi - lo + 1))
        bases = np.cumsum([0] + [kc for _, kc in kwin])
        noh = int(bases[-1])

        wh = np.ascontiguousarray(
            w_np.reshape(KB, 128, P).transpose(1, 0, 2).reshape(128, KB * P)
        ).astype(bf16)
        bias_rep = np.ascontiguousarray(
            np.broadcast_to(b_np[None, :], (128, P)), dtype=np.float32)
        in_maps, ids_all = [], []
        for bb in range(B):
            rows, streams = percore[bb]
            ids_all.append(streams)
            oh_np = np.zeros((128, noh, 128), np.float32)
            for st in range(NST):
                ids, cr = streams[st]
                n = len(ids)
                for t in range(QT):
                    j = st * QT + t
                    kbase, kcnt = kwin[j]
                    seg = cr[t * 128:min(n, (t + 1) * 128)]
                    q = np.arange(len(seg))
                    oh_np[seg % 128, bases[j] + seg // 128 - kbase, q] = 1.0
            xc = np.zeros((HROWS, D), np.float32)
            xc[:len(rows)] = x_full[bb][rows]
            xr = xc.reshape(SCH, SCHW, KB, 128).transpose(3, 0, 2, 1) \
                .reshape(128, SCH * KB * SCHW)
            in_maps.append({
                "xh": np.ascontiguousarray(xr).astype(bf16),
                "wh": wh,
                "bias": bias_rep,
                "oh": np.ascontiguousarray(
                    oh_np.reshape(128, noh * 128)).astype(bf16),
            })
    except ValueError:
        res1, res2 = _numpy_ref(flag, x_full, s1, e1, qb, s2, e2, w_np, b_np)
        return np.asarray(res1, np.float32), np.asarray(res2, np.float32)

    nc = _get_nc(tuple(kwin))
    out = run_bass_kernel_spmd(nc, in_maps, core_ids=list(range(NCORES)))
    _cache["last_run"] = out

    res1 = np.zeros((NQ, 2 * P), np.float32)
    res2 = np.zeros((NQ, 2 * P), np.float32)
    for bb in range(B):
        streams = ids_all[bb]
        rr1 = np.asarray(out.results[bb]["r1"]).astype(np.float32)
        rr2 = np.asarray(out.results[bb]["r2"]).astype(np.float32)
        for st, (res, rr) in enumerate([(res1, rr1), (res1, rr1),
                                        (res2, rr2), (res2, rr2)]):
            ids, _ = streams[st]
            endp = st % 2
            n = len(ids)
            if n:
                res[ids, endp * P:(endp + 1) * P] = \
                    rr[:n, endp * P:(endp + 1) * P]
    return res1, res2
